# revision 62
# baseline (speedup 1.0000x reference)
"""Trainium2 Bass kernel for the 2-bit-DoReFa quantized BasicBlock.

  out = conv3x3(q(bn2(conv3x3(q(bn1(x)), Wq1))), Wq2) + x
  q(h) = round(3*clip(relu(h),0,1))/3,  Wq = DoReFa-2bit(w) in {-1,-1/3,1/3,1}

Sharding: data-parallel over batch, 4 images per NeuronCore x 8 cores;
conv weights and BN parameters replicated.

Per-core kernel design:
  * Quantized activations/weights are exact small integers when scaled by 3:
    a3 in {0..3}, w3 in {-3,-1,1,3}.  Activations are stored with a +12
    offset (a' = a3+12 in {12..15}): in that fp8e4 range the lattice spacing
    is exactly 1, so the fp8 cast itself performs round-to-nearest-even --
    the separate rounding pass disappears.  Padding is memset to 12 (== 0+12)
    so the offset is spatially uniform; the resulting per-channel constant
    12*sum(W) folds into the next stage's BN bias (conv1) or the final
    rescale bias (conv2).
  * Each 3x3 conv is 9 accumulating DoubleRow 128x(2x128) matmuls per output
    tile with exact integer accumulation in fp32 PSUM; the 1/9 rescale and
    offset corrections fold into the epilogue affines.
  * The aq buffers store the two cin-halves interleaved per column
    ([p, (col, blk)]): Tile's dependency tracking is byte-interval based, so
    this keeps each conv tile's rhs read interval compact and the tile gates
    only on the quant units that wrote its rows (a stride-2 moving dim runs
    at the same 1 col/cycle PE rate, HW-verified).
  * Stage-1 quant is two DVE ops (bit-exact fp32 affine w/ folded +12 offset,
    then clamp(12,15) -> fp8 cast-round); stage-2 quant is one ACT op
    (Relu(ps*scale+bias)) + one DVE clamp; final combine is ACT
    Identity(ps*(1/9)+corr) + DVE residual add.  The vector work is split so
    the ACT queue holds only PSUM consumers (paced by the PE) and every
    engine stays far below the PE's runtime -- which also avoids the P0
    power downclock (2.0GHz) that throttled heavier-vector variants.
  * DMA: the per-core startup feed is bandwidth-bound (~200-280GB/s;
    concurrent hardware rings split the same engine pool, and the gpsimd
    ring is software-DGE at ~20GB/s -- data never goes there).  Everything
    latency-critical therefore rides the sync ring in ONE FIFO: prm, w1-cb0,
    x chunk 0, w1-cb1, then chunks 1-3 and images 1-3 on ordered chains;
    only w2 uses the scalar ring, dep-held until the x feed clears.  Image-0
    stage-1 quant chases the chunk-0 DMA halves (half-chunk units, chunk 2's
    affines on the startup-idle ACT engine), so the stream starts ~15.8us in
    and runs gapless: the FIFO keeps later chunks just ahead of the
    tile-pair consumption rate.
  * HAM: the PE demotes to half rate (K=4/8) when a ~3.4us window sees low
    duty.  Warmup matmuls on a DVE-memset scratch tile ramp K to 8/8 during
    the DMA window so the 1008-matmul stream runs at full rate
    (~193.5ns/matmul, 456 cols) once the early feed-gated tiles clear.
  * Tail: the final cout block ends on a 2-row tile (split epilogues, out
    DMA alternating sync/scalar rings), so the exposed post-stream chain is
    one short ACT+add+DMA before the fixed ~3.7us teardown barrier.
"""
import os
from contextlib import ExitStack

import numpy as np

import concourse.bacc as bacc
import concourse.tile as tile
from concourse import mybir
from concourse.bass_utils import run_bass_kernel_spmd

F32 = mybir.dt.float32
OP = mybir.AluOpType
AF = mybir.ActivationFunctionType

N_CORES = 8
N_IMG = 4
C = 256
H = W = 56
PW = W + 1
NPIX = H * W
RT = 8
NT = H // RT
TQ = RT * PW                                   # 456
NPAD = ((PW * (H + 2) + 2 + 15) // 16) * 16    # 3312
N_CHUNK = 4
CR = H // N_CHUNK
ACT_DT = mybir.dt.float8e4
X_DT = mybir.dt.float32   # x must ship fp32: fp16 quant-path rounding
                          # cascades through the two quant stages (measured
                          # rel err 2.7e-2 > the 2e-2 gate)
N_WARMUP = 17
AQ_INTERLEAVED = True   # aq layout [p, (col,blk)] -> compact per-tile read
                        # intervals, so conv tiles gate on just their chunks

LAST_EXEC_NS = None          # set when BASS_TRACE=1
_CACHED = {}


def _build():
    nc = bacc.Bacc("TRN2", target_bir_lowering=False, debug=False)

    x_d = nc.dram_tensor("x", [N_IMG, C, H, W], X_DT, kind="ExternalInput")
    w1_d = nc.dram_tensor("w1t", [128, 4608], ACT_DT, kind="ExternalInput")
    w2_d = nc.dram_tensor("w2t", [128, 4608], ACT_DT, kind="ExternalInput")
    prm_d = nc.dram_tensor("prm", [128, 12], F32, kind="ExternalInput")
    out_d = nc.dram_tensor("out", [N_IMG, C, H, W], F32, kind="ExternalOutput")

    xr = x_d.ap().rearrange("n (b k) h w -> n k b (h w)", b=2)
    outr = out_d.ap().rearrange("n (b k) h w -> n k b (h w)", b=2)

    with tile.TileContext(nc) as tc, ExitStack() as ctx:
        wpool = ctx.enter_context(tc.tile_pool(name="wpool", bufs=1))
        xpool = ctx.enter_context(tc.tile_pool(name="xpool", bufs=4))
        aqpool = ctx.enter_context(tc.tile_pool(name="aqpool", bufs=1))
        t1pool = ctx.enter_context(tc.tile_pool(name="t1pool", bufs=3))
        t2pool = ctx.enter_context(tc.tile_pool(name="t2pool", bufs=6))
        t3pool = ctx.enter_context(tc.tile_pool(name="t3pool", bufs=6))
        pspool = ctx.enter_context(tc.tile_pool(name="pspool", bufs=8,
                                                space="PSUM"))

        # PE warmup scratch: DVE memset (no DMA or gpsimd dependency), so the
        # p-state/HAM ramp starts as early as the engine prologue allows.
        # Only the LDWEIGHTS-read prefix is initialized; the rest of the
        # matmul rhs window reads whatever is in SBUF (scratch PSUM dst).
        wu_src = aqpool.tile([128, 2, 512], ACT_DT)
        nc.vector.memset(wu_src[:, :, 0:130], 1.0)

        # prm + both weights ride the scalar hardware ring (w1 split so the
        # cb=0 half lands first); x chunk-0 halves head the sync ring.  The
        # gpsimd DMA path is software-DGE (~20GB/s) -- never put data there.
        prm = wpool.tile([128, 12], F32)
        w1_sb = wpool.tile([128, 4608], ACT_DT)
        w2_sb = wpool.tile([128, 4608], ACT_DT)

        x_sbs = [None] * N_IMG

        def x_alloc(img):
            x_sb = xpool.tile([128, 2, NPIX], X_DT, tag="x", name=f"x_{img}")
            x_sbs[img] = x_sb

        # Fixed ping-pong padded activation buffers; borders memset to 12
        # (= quant offset for a=0) once -- interior writes never touch them.
        # AQ_INTERLEAVED stores the two cin-halves interleaved per column
        # ([p, (col, blk)]): a conv tile's rhs then reads a compact byte
        # interval, so its dependencies cover only the rows it touches.
        aq1s, aq2s = [], []
        for i in range(2):
            if AQ_INTERLEAVED:
                a1 = aqpool.tile([128, 2 * NPAD], ACT_DT, name=f"aq1_{i}", tag=f"aq1_{i}")
                a2 = aqpool.tile([128, 2 * NPAD], ACT_DT, name=f"aq2_{i}", tag=f"aq2_{i}")
            else:
                a1 = aqpool.tile([128, 2, NPAD], ACT_DT, name=f"aq1_{i}", tag=f"aq1_{i}")
                a2 = aqpool.tile([128, 2, NPAD], ACT_DT, name=f"aq2_{i}", tag=f"aq2_{i}")
            aq1s.append(a1)
            aq2s.append(a2)
            for a in (a1, a2):
                if AQ_INTERLEAVED:
                    nc.gpsimd.memset(a[:, 0:2 * (PW + 1)], 12.0)
                    mid = a[:, 2 * (PW + W + 1): 2 * (PW + W + 1 + (H - 1) * PW)]
                    mid3 = mid.rearrange("p (r c) -> p r c", c=2 * PW)
                    nc.gpsimd.memset(mid3[:, :, 0:2 * (PW - W)], 12.0)
                    nc.gpsimd.memset(a[:, 2 * (H * PW + W + 1): 2 * NPAD], 12.0)
                else:
                    for blk in range(2):
                        nc.gpsimd.memset(a[:, blk, 0:PW + 1], 12.0)
                        mid = a[:, blk, PW + W + 1: PW + W + 1 + (H - 1) * PW]
                        mid3 = mid.rearrange("p (r c) -> p r c", c=PW)[:, :, 0:PW - W]
                        nc.gpsimd.memset(mid3, 12.0)
                        nc.gpsimd.memset(a[:, blk, H * PW + W + 1: NPAD], 12.0)

        def aq_dst(aq, blk, rows, y0):
            """Interior write view [p, rows, W] for quant output."""
            lo = (y0 + 1) * PW + 1
            if AQ_INTERLEAVED:
                A = aq[:].rearrange("p (n j) -> p n j", j=2)
                d = A[:, lo: lo + rows * PW, blk]
            else:
                d = aq[:, blk, lo: lo + rows * PW]
            return d.rearrange("p (r c) -> p r c", c=PW)[:, :, 0:W]

        def aq_rhs(aq, t, tap, row0=None, nrows=RT):
            """Matmul rhs [p, 2, nrows, W] for tile t (or row range), tap.
            The per-row pad column is skipped via the 4D AP, so the matmul
            computes only the 448 real output columns per 8-row tile."""
            ky, kx = divmod(tap, 3)
            if row0 is None:
                row0 = t * RT
            off = row0 * PW + ky * PW + kx
            if AQ_INTERLEAVED:
                win = aq[:, 2 * off: 2 * (off + nrows * PW)]
                return win.rearrange("p (r c j) -> p j r c", j=2,
                                     c=PW)[:, :, :, 0:W]
            return aq[:, :, off: off + nrows * PW]

        # --- DMA ordering: a single dma_start tops out at ~175GB/s, so run
        # three concurrent chains (two for image chunks, one for weights),
        # each internally ordered so early data still lands first.
        last_dma = [None, None, None]

        def chain(cidx, inst):
            # add_dep_helper(dependent, prerequisite): inst waits chain tail.
            if last_dma[cidx] is not None:
                tile.add_dep_helper(inst.ins, last_dma[cidx].ins, sync=True,
                                    reason="serialize DMA chain")
            last_dma[cidx] = inst
            return inst

        def x_chunk_dma(img, ch):
            base = ch * CR * W
            q = CR * W // 2
            for h in range(2):
                sl = slice(base + h * q, base + (h + 1) * q)
                chain(h, nc.sync.dma_start(x_sbs[img][:, :, sl],
                                           xr[img][:, :, sl]))

        def stage1_dma(img):
            x_alloc(img)
            for ch in range(N_CHUNK):
                x_chunk_dma(img, ch)

        def quant1_stage(src_ap, aq, blk, s_col, c_col, rows, y0):
            """Stage-1 quant, DVE only (bit-exact fp32 affine):
            a' = cast_fp8(min(max(src*s + coff, 12), 15)), coff = 3*bias+12;
            the fp8 cast in [12,15] rounds half-even == jnp.round."""
            t = t1pool.tile([128, rows * W], F32, tag="q1tmp")
            t3 = t[:].rearrange("p (r c) -> p r c", c=W)
            nc.vector.tensor_scalar(t3, src_ap, prm[:, s_col:s_col + 1],
                                    prm[:, c_col:c_col + 1], OP.mult, OP.add)
            nc.vector.tensor_scalar(aq_dst(aq, blk, rows, y0), t3,
                                    12.0, 15.0, OP.max, OP.min)

        def quant2_stage(src_ap, aq, blk, s_col, c_col, rows, y0):
            """Stage-2 quant: ACT affine+Relu (PSUM consumer) + DVE clamp."""
            t = t2pool.tile([128, rows * W], F32, tag="q2tmp")
            t3 = t[:].rearrange("p (r c) -> p r c", c=W)
            nc.scalar.activation(t3, src_ap, AF.Relu,
                                 scale=prm[:, s_col:s_col + 1],
                                 bias=prm[:, c_col:c_col + 1])
            nc.vector.tensor_scalar(aq_dst(aq, blk, rows, y0), t3,
                                    3.0, 12.0, OP.min, OP.add)

        def conv_tile(aq, w_sb, t, cb, row0=None, nrows=RT):
            ps = pspool.tile([128, nrows * W], F32, tag="ps")
            w5 = w_sb[:].rearrange("p (cb t j m) -> p cb t j m", cb=2, t=9,
                                   j=2)
            for tap in range(9):
                lhsT = w5[:, cb, tap]
                nc.tensor.matmul(ps[:], lhsT,
                                 aq_rhs(aq, t, tap, row0, nrows),
                                 perf_mode=mybir.MatmulPerfMode.DoubleRow,
                                 start=(tap == 0), stop=(tap == 8))
            return ps

        def stage1_units(img):
            """Quant thunks for image img, one per (chunk, blk)."""
            aq1 = aq1s[img % 2]
            x_sb = x_sbs[img]

            def make(ch, blk):
                def run():
                    sl = slice(ch * CR * W, (ch + 1) * CR * W)
                    src = x_sb[:, blk, sl].rearrange("p (r c) -> p r c", c=W)
                    quant1_stage(src, aq1, blk, 0 + blk, 2 + blk, CR, ch * CR)
                return run
            return [make(ch, blk) for ch in range(N_CHUNK) for blk in range(2)]

        def stage1_img0():
            """Image-0 stage-1 quant, latency-choreographed across ACT+DVE:
            ch0 h-split with the b1 affine on ACT (b0 on DVE), ch1/ch3
            h-split with both affines on ACT (a' = min(max(x*s+3b,0),3)+12,
            Relu on ACT then (min,add) on DVE -- the quant2 split); ch2 runs
            as full DVE units.  Both engines chase the DMA halves so the
            first tiles' aq rows are ready as the warmup matmuls end."""
            aq1, x_sb = aq1s[0], x_sbs[0]
            hh = CR // 2

            def src_of(ch, blk, r0, nr):
                sl = slice((ch * CR + r0) * W, (ch * CR + r0 + nr) * W)
                return x_sb[:, blk, sl].rearrange("p (r c) -> p r c", c=W)

            def act_aff(ch, blk, r0, nr):
                t = t2pool.tile([128, nr * W], F32, tag="q1a")
                t3 = t[:].rearrange("p (r c) -> p r c", c=W)
                nc.scalar.activation(t3, src_of(ch, blk, r0, nr), AF.Relu,
                                     scale=prm[:, 0 + blk:1 + blk],
                                     bias=prm[:, 10 + blk:11 + blk])
                return t3

            def dve_aff(ch, blk, r0, nr):
                t = t1pool.tile([128, nr * W], F32, tag="q1tmp")
                t3 = t[:].rearrange("p (r c) -> p r c", c=W)
                nc.vector.tensor_scalar(t3, src_of(ch, blk, r0, nr),
                                        prm[:, 0 + blk:1 + blk],
                                        prm[:, 2 + blk:3 + blk],
                                        OP.mult, OP.add)
                return t3

            def cl_f(t3, ch, blk, r0, nr):   # after DVE affine (+12 folded)
                nc.vector.tensor_scalar(aq_dst(aq1, blk, nr, ch * CR + r0),
                                        t3, 12.0, 15.0, OP.max, OP.min)

            def cl_r(t3, ch, blk, r0, nr):   # after ACT Relu affine
                nc.vector.tensor_scalar(aq_dst(aq1, blk, nr, ch * CR + r0),
                                        t3, 3.0, 12.0, OP.min, OP.add)

            # NOTE: the Tile scheduler reorders within queues, so DVE clamps
            # gated on ACT affines get hoisted unpredictably -- keep ch0/ch1
            # (first-tile critical) purely on DVE; only ch2 (which lands on
            # the scalar ring while DVE is busy) uses the ACT-affine split.
            for ch in (0, 1):
                for r0 in (0, hh):
                    for blk in range(2):
                        a0 = dve_aff(ch, blk, r0, hh)
                        cl_f(a0, ch, blk, r0, hh)
            for r0 in (0, hh):               # ch2: ACT affines, DVE clamps
                a0 = act_aff(2, 0, r0, hh)
                a1 = act_aff(2, 1, r0, hh)
                cl_r(a0, 2, 0, r0, hh)
                cl_r(a1, 2, 1, r0, hh)
            for r0 in (0, hh):               # ch3: full DVE half-units
                for blk in range(2):
                    a0 = dve_aff(3, blk, r0, hh)
                    cl_f(a0, 3, blk, r0, hh)

        def conv1_tile(img, t, cb):
            aq1, aq2 = aq1s[img % 2], aq2s[img % 2]
            ps = conv_tile(aq1, w1_sb, t, cb)
            psv = ps[:].rearrange("p (r c) -> p r c", c=W)
            quant2_stage(psv, aq2, cb, 4 + cb, 6 + cb, RT, t * RT)

        def conv2_tile(img, t, cb, split_epi=False):
            aq2, x_sb = aq2s[img % 2], x_sbs[img]
            ps = conv_tile(aq2, w2_sb, t, cb)
            psa = ps[:].rearrange("p (r c) -> p r c", c=W)
            # The very last tiles split their epilogue in half-row batches
            # (second half's out DMA on the scalar ring) to shorten the
            # post-final-matmul critical chain.
            nh = 2 if split_epi else 1
            rh = RT // nh
            for h in range(nh):
                psv = psa[:, h * rh:(h + 1) * rh]
                tt = t3pool.tile([128, rh * W], F32, tag="ot",
                                 name=f"ot_{img}_{t}_{cb}_{h}")
                tt3 = tt[:].rearrange("p (r c) -> p r c", c=W)
                nc.scalar.activation(tt3, psv, AF.Identity, scale=1.0 / 9.0,
                                     bias=prm[:, 8 + cb:9 + cb])
                lo = t * RT * W + h * rh * W
                res = x_sb[:, cb, lo: lo + rh * W]
                res3 = res.rearrange("p (r c) -> p r c", c=W)
                nc.vector.tensor_tensor(tt3, tt3, res3, OP.add)
                oq = nc.scalar if (split_epi and h % 2 == 1) else nc.sync
                oq.dma_start(outr[img][:, cb, lo: lo + rh * W], tt[:])

        def conv1_img(img, interleave=()):
            # Spread the next image's stage-1 units between conv1 tiles so
            # the DVE/ACT queues never head-of-line block the conv epilogues.
            inter = list(interleave)
            for t in range(NT):
                for cb in range(2):
                    conv1_tile(img, t, cb)
                for _ in range(2):
                    if inter:
                        inter.pop(0)()
            for f in inter:
                f()

        def conv2_rows(img, row0, nrows, cb, oq):
            """Row-range conv2 tile + one-shot epilogue, out DMA on oq."""
            aq2, x_sb = aq2s[img % 2], x_sbs[img]
            ps = conv_tile(aq2, w2_sb, None, cb, row0=row0, nrows=nrows)
            psv = ps[:].rearrange("p (r c) -> p r c", c=W)
            tt = t3pool.tile([128, nrows * W], F32, tag="ot",
                             name=f"otr_{img}_{row0}_{cb}")
            tt3 = tt[:].rearrange("p (r c) -> p r c", c=W)
            nc.scalar.activation(tt3, psv, AF.Identity, scale=1.0 / 9.0,
                                 bias=prm[:, 8 + cb:9 + cb])
            lo = row0 * W
            res3 = x_sb[:, cb, lo: lo + nrows * W].rearrange(
                "p (r c) -> p r c", c=W)
            nc.vector.tensor_tensor(tt3, tt3, res3, OP.add)
            oq.dma_start(outr[img][:, cb, lo: lo + nrows * W], tt[:])

        def conv2_img(img, last=False):
            for t in range(NT):
                for cb in range(2):
                    if last and t == NT - 1 and cb == 1:
                        # final cout block ends on a 2-row tile so the
                        # exposed post-stream epilogue chain is minimal
                        conv2_rows(img, (NT - 1) * RT, RT - 2, 1, nc.sync)
                        conv2_rows(img, H - 2, 2, 1, nc.scalar)
                    else:
                        conv2_tile(img, t, cb,
                                   split_epi=(last and t == NT - 1))

        # --- startup: concurrent rings split the same engine bandwidth, so
        # everything latency-critical rides the sync ring in one FIFO with
        # prm + the w1 halves at its head (the v0 all-on-sync feed held a
        # steady ~2.9us/chunk x cadence; split-ring variants degraded it).
        # Only w2 uses the scalar ring, dep-held until chunk 2 has landed.
        nc.sync.dma_start(prm[:], prm_d.ap())
        x_alloc(0)
        x_chunk_dma(0, 0)
        # both w1 halves slot in after chunk 0: the stream gate is chunk-0's
        # h1 half + its quant, and w1-cb0 still lands before the first taps
        # can issue; cumulative bytes ahead of chunk 1+ are unchanged.
        nc.sync.dma_start(w1_sb[:, 0:2304], w1_d.ap()[:, 0:2304])
        nc.sync.dma_start(w1_sb[:, 2304:4608], w1_d.ap()[:, 2304:4608])
        for ch in range(1, N_CHUNK):
            x_chunk_dma(0, ch)
        w2i = nc.scalar.dma_start(w2_sb[:], w2_d.ap())
        tile.add_dep_helper(w2i.ins, last_dma[1].ins, sync=True,
                            reason="hold w2 until the startup x feed clears")

        wu_ps = pspool.tile([128, TQ], F32, tag="ps")
        for i in range(N_WARMUP):
            nc.tensor.matmul(wu_ps[:], wu_src[:, :, 0:128],
                             wu_src[:, :, 0:TQ],
                             perf_mode=mybir.MatmulPerfMode.DoubleRow,
                             start=(i == 0), stop=(i == N_WARMUP - 1))

        stage1_img0()
        if N_IMG > 1:
            stage1_dma(1)
            conv1_img(0, interleave=stage1_units(1))
        else:
            conv1_img(0)
        for img in range(1, N_IMG):
            nxt = ()
            if img + 1 < N_IMG:
                stage1_dma(img + 1)
                nxt = stage1_units(img + 1)
            conv1_img(img, interleave=nxt)
            conv2_img(img - 1)
        conv2_img(N_IMG - 1, last=True)

    nc.compile()
    return nc


def _host_prep(w1, w2, g1, b1, m1, v1, g2, b2, m2, v2):
    """BN folds + DoReFa weight quantization, replicating the reference's
    fp32 op sequence exactly (jax CPU), then weight layout transforms."""
    import jax
    import jax.numpy as jnp
    import ml_dtypes

    cpu = jax.local_devices(backend="cpu")[0]
    with jax.default_device(cpu):
        eps = jnp.float32(1e-5)
        inv1 = g1 / jnp.sqrt(v1 + eps)
        bias1 = b1 - m1 * inv1
        inv2 = g2 / jnp.sqrt(v2 + eps)
        bias2 = b2 - m2 * inv2

        def wq3(w):
            wt = jnp.tanh(w)
            wn = wt / (2.0 * jnp.max(jnp.abs(wt))) + 0.5
            return 2.0 * jnp.round(wn * 3.0) - 3.0   # exact ints {-3,-1,1,3}

        wq1 = np.asarray(wq3(jnp.asarray(w1)), dtype=np.float32)
        wq2 = np.asarray(wq3(jnp.asarray(w2)), dtype=np.float32)
        inv1, bias1, inv2, bias2 = (np.asarray(a, dtype=np.float32)
                                    for a in (inv1, bias1, inv2, bias2))

    S1 = wq1.reshape(256, -1).sum(axis=1).astype(np.float32)
    S2 = wq2.reshape(256, -1).sum(axis=1).astype(np.float32)

    s1 = 3.0 * inv1
    c1 = 3.0 * bias1 + 12.0      # stage-1 offset folded into the affine
    s2 = inv2 / np.float32(3.0)
    c2 = 3.0 * bias2 - 4.0 * S1 * inv2
    corr2 = -(np.float32(4.0) / np.float32(3.0)) * S2

    def wlayout(wq):
        # [cout, cin, ky, kx] -> [k(128), cb(2), tap(9), blk(2), m(128)]
        a = wq.reshape(2, 128, 2, 128, 9)             # cb, m, blk, k, tap
        return np.ascontiguousarray(np.transpose(a, (3, 0, 4, 2, 1))
                                    .reshape(128, 4608)
                                    ).astype(ml_dtypes.float8_e4m3)

    c1m12 = 3.0 * bias1                 # c1 - 12, for the ACT-Relu variant

    prm = np.zeros((128, 12), np.float32)
    for col, v in enumerate((s1, c1, s2, c2, corr2, c1m12)):
        prm[:, 2 * col] = v[0:128]
        prm[:, 2 * col + 1] = v[128:256]

    return {"w1t": wlayout(wq1), "w2t": wlayout(wq2), "prm": prm}


def kernel(x, w1, w2, g1, b1, m1, v1, g2, b2, m2, v2):
    global LAST_EXEC_NS
    x = np.asarray(x, dtype=np.float32)

    if "nc" not in _CACHED:
        _CACHED["nc"] = _build()
    nc = _CACHED["nc"]

    shared = _host_prep(w1, w2, g1, b1, m1, v1, g2, b2, m2, v2)
    in_maps = []
    for c in range(N_CORES):
        m = dict(shared)
        m["x"] = x[N_IMG * c:N_IMG * (c + 1)]
        in_maps.append(m)

    trace = bool(int(os.environ.get("BASS_TRACE", "0")))
    res = run_bass_kernel_spmd(nc, in_maps, core_ids=list(range(N_CORES)),
                               trace=trace)
    LAST_EXEC_NS = res.exec_time_ns
    return np.concatenate([res.results[c]["out"] for c in range(N_CORES)],
                          axis=0)



# revision 63
# speedup vs baseline: 1.0006x; 1.0006x over previous
"""Trainium2 Bass kernel for the 2-bit-DoReFa quantized BasicBlock.

  out = conv3x3(q(bn2(conv3x3(q(bn1(x)), Wq1))), Wq2) + x
  q(h) = round(3*clip(relu(h),0,1))/3,  Wq = DoReFa-2bit(w) in {-1,-1/3,1/3,1}

Sharding: data-parallel over batch, 4 images per NeuronCore x 8 cores;
conv weights and BN parameters replicated.

Per-core kernel design:
  * Quantized activations/weights are exact small integers when scaled by 3:
    a3 in {0..3}, w3 in {-3,-1,1,3}.  Activations are stored with a +12
    offset (a' = a3+12 in {12..15}): in that fp8e4 range the lattice spacing
    is exactly 1, so the fp8 cast itself performs round-to-nearest-even --
    the separate rounding pass disappears.  Padding is memset to 12 (== 0+12)
    so the offset is spatially uniform; the resulting per-channel constant
    12*sum(W) folds into the next stage's BN bias (conv1) or the final
    rescale bias (conv2).
  * Each 3x3 conv is 9 accumulating DoubleRow 128x(2x128) matmuls per output
    tile with exact integer accumulation in fp32 PSUM; the 1/9 rescale and
    offset corrections fold into the epilogue affines.
  * The aq buffers store the two cin-halves interleaved per column
    ([p, (col, blk)]): Tile's dependency tracking is byte-interval based, so
    this keeps each conv tile's rhs read interval compact and the tile gates
    only on the quant units that wrote its rows (a stride-2 moving dim runs
    at the same 1 col/cycle PE rate, HW-verified).
  * Stage-1 quant is two DVE ops (bit-exact fp32 affine w/ folded +12 offset,
    then clamp(12,15) -> fp8 cast-round); stage-2 quant is one ACT op
    (Relu(ps*scale+bias)) + one DVE clamp; final combine is ACT
    Identity(ps*(1/9)+corr) + DVE residual add.  The vector work is split so
    the ACT queue holds only PSUM consumers (paced by the PE) and every
    engine stays far below the PE's runtime -- which also avoids the P0
    power downclock (2.0GHz) that throttled heavier-vector variants.
  * DMA: the per-core startup feed is bandwidth-bound (~200-280GB/s;
    concurrent hardware rings split the same engine pool, and the gpsimd
    ring is software-DGE at ~20GB/s -- data never goes there).  Everything
    latency-critical therefore rides the sync ring in ONE FIFO: prm, w1-cb0,
    x chunk 0, w1-cb1, then chunks 1-3 and images 1-3 on ordered chains;
    only w2 uses the scalar ring, dep-held until the x feed clears.  Image-0
    stage-1 quant chases the chunk-0 DMA halves (half-chunk units, chunk 2's
    affines on the startup-idle ACT engine), so the stream starts ~15.8us in
    and runs gapless: the FIFO keeps later chunks just ahead of the
    tile-pair consumption rate.
  * HAM: the PE demotes to half rate (K=4/8) when a ~3.4us window sees low
    duty.  Warmup matmuls on a DVE-memset scratch tile ramp K to 8/8 during
    the DMA window so the 1008-matmul stream runs at full rate
    (~193.5ns/matmul, 456 cols) once the early feed-gated tiles clear.
  * Tail: the final cout block ends on a 2-row tile (split epilogues, out
    DMA alternating sync/scalar rings), so the exposed post-stream chain is
    one short ACT+add+DMA before the fixed ~3.7us teardown barrier.
"""
import os
from contextlib import ExitStack

import numpy as np

import concourse.bacc as bacc
import concourse.tile as tile
from concourse import mybir
from concourse.bass_utils import run_bass_kernel_spmd

F32 = mybir.dt.float32
OP = mybir.AluOpType
AF = mybir.ActivationFunctionType

N_CORES = 8
N_IMG = 4
C = 256
H = W = 56
PW = W + 1
NPIX = H * W
RT = 8
NT = H // RT
TQ = RT * PW                                   # 456
NPAD = ((PW * (H + 2) + 2 + 15) // 16) * 16    # 3312
N_CHUNK = 4
CR = H // N_CHUNK
ACT_DT = mybir.dt.float8e4
X_DT = mybir.dt.float32   # x must ship fp32: fp16 quant-path rounding
                          # cascades through the two quant stages (measured
                          # rel err 2.7e-2 > the 2e-2 gate)
N_WARMUP = 17
AQ_INTERLEAVED = True   # aq layout [p, (col,blk)] -> compact per-tile read
                        # intervals, so conv tiles gate on just their chunks

LAST_EXEC_NS = None          # set when BASS_TRACE=1
_CACHED = {}


def _build():
    nc = bacc.Bacc("TRN2", target_bir_lowering=False, debug=False)

    x_d = nc.dram_tensor("x", [N_IMG, C, H, W], X_DT, kind="ExternalInput")
    w1_d = nc.dram_tensor("w1t", [128, 4608], ACT_DT, kind="ExternalInput")
    w2_d = nc.dram_tensor("w2t", [128, 4608], ACT_DT, kind="ExternalInput")
    prm_d = nc.dram_tensor("prm", [128, 12], F32, kind="ExternalInput")
    out_d = nc.dram_tensor("out", [N_IMG, C, H, W], F32, kind="ExternalOutput")

    xr = x_d.ap().rearrange("n (b k) h w -> n k b (h w)", b=2)
    outr = out_d.ap().rearrange("n (b k) h w -> n k b (h w)", b=2)

    with tile.TileContext(nc) as tc, ExitStack() as ctx:
        wpool = ctx.enter_context(tc.tile_pool(name="wpool", bufs=1))
        xpool = ctx.enter_context(tc.tile_pool(name="xpool", bufs=4))
        aqpool = ctx.enter_context(tc.tile_pool(name="aqpool", bufs=1))
        t1pool = ctx.enter_context(tc.tile_pool(name="t1pool", bufs=3))
        t2pool = ctx.enter_context(tc.tile_pool(name="t2pool", bufs=6))
        t3pool = ctx.enter_context(tc.tile_pool(name="t3pool", bufs=6))
        pspool = ctx.enter_context(tc.tile_pool(name="pspool", bufs=8,
                                                space="PSUM"))

        # PE warmup scratch: DVE memset (no DMA or gpsimd dependency), so the
        # p-state/HAM ramp starts as early as the engine prologue allows.
        # Only the LDWEIGHTS-read prefix is initialized; the rest of the
        # matmul rhs window reads whatever is in SBUF (scratch PSUM dst).
        wu_src = aqpool.tile([128, 2, 512], ACT_DT)
        nc.vector.memset(wu_src[:, :, 0:130], 1.0)

        # prm + both weights ride the scalar hardware ring (w1 split so the
        # cb=0 half lands first); x chunk-0 halves head the sync ring.  The
        # gpsimd DMA path is software-DGE (~20GB/s) -- never put data there.
        prm = wpool.tile([128, 12], F32)
        w1_sb = wpool.tile([128, 4608], ACT_DT)
        w2_sb = wpool.tile([128, 4608], ACT_DT)

        x_sbs = [None] * N_IMG

        def x_alloc(img):
            x_sb = xpool.tile([128, 2, NPIX], X_DT, tag="x", name=f"x_{img}")
            x_sbs[img] = x_sb

        # Fixed ping-pong padded activation buffers; borders memset to 12
        # (= quant offset for a=0) once -- interior writes never touch them.
        # AQ_INTERLEAVED stores the two cin-halves interleaved per column
        # ([p, (col, blk)]): a conv tile's rhs then reads a compact byte
        # interval, so its dependencies cover only the rows it touches.
        aq1s, aq2s = [], []
        for i in range(2):
            if AQ_INTERLEAVED:
                a1 = aqpool.tile([128, 2 * NPAD], ACT_DT, name=f"aq1_{i}", tag=f"aq1_{i}")
                a2 = aqpool.tile([128, 2 * NPAD], ACT_DT, name=f"aq2_{i}", tag=f"aq2_{i}")
            else:
                a1 = aqpool.tile([128, 2, NPAD], ACT_DT, name=f"aq1_{i}", tag=f"aq1_{i}")
                a2 = aqpool.tile([128, 2, NPAD], ACT_DT, name=f"aq2_{i}", tag=f"aq2_{i}")
            aq1s.append(a1)
            aq2s.append(a2)
            for a in (a1, a2):
                if AQ_INTERLEAVED:
                    nc.gpsimd.memset(a[:, 0:2 * (PW + 1)], 12.0)
                    mid = a[:, 2 * (PW + W + 1): 2 * (PW + W + 1 + (H - 1) * PW)]
                    mid3 = mid.rearrange("p (r c) -> p r c", c=2 * PW)
                    nc.gpsimd.memset(mid3[:, :, 0:2 * (PW - W)], 12.0)
                    nc.gpsimd.memset(a[:, 2 * (H * PW + W + 1): 2 * NPAD], 12.0)
                else:
                    for blk in range(2):
                        nc.gpsimd.memset(a[:, blk, 0:PW + 1], 12.0)
                        mid = a[:, blk, PW + W + 1: PW + W + 1 + (H - 1) * PW]
                        mid3 = mid.rearrange("p (r c) -> p r c", c=PW)[:, :, 0:PW - W]
                        nc.gpsimd.memset(mid3, 12.0)
                        nc.gpsimd.memset(a[:, blk, H * PW + W + 1: NPAD], 12.0)

        def aq_dst(aq, blk, rows, y0):
            """Interior write view [p, rows, W] for quant output."""
            lo = (y0 + 1) * PW + 1
            if AQ_INTERLEAVED:
                A = aq[:].rearrange("p (n j) -> p n j", j=2)
                d = A[:, lo: lo + rows * PW, blk]
            else:
                d = aq[:, blk, lo: lo + rows * PW]
            return d.rearrange("p (r c) -> p r c", c=PW)[:, :, 0:W]

        def aq_rhs(aq, t, tap, row0=None, nrows=RT):
            """Matmul rhs [p, 2, nrows, W] for tile t (or row range), tap.
            The per-row pad column is skipped via the 4D AP, so the matmul
            computes only the 448 real output columns per 8-row tile."""
            ky, kx = divmod(tap, 3)
            if row0 is None:
                row0 = t * RT
            off = row0 * PW + ky * PW + kx
            if AQ_INTERLEAVED:
                win = aq[:, 2 * off: 2 * (off + nrows * PW)]
                return win.rearrange("p (r c j) -> p j r c", j=2,
                                     c=PW)[:, :, :, 0:W]
            return aq[:, :, off: off + nrows * PW]

        # --- DMA ordering: a single dma_start tops out at ~175GB/s, so run
        # three concurrent chains (two for image chunks, one for weights),
        # each internally ordered so early data still lands first.
        last_dma = [None, None, None]

        def chain(cidx, inst):
            # add_dep_helper(dependent, prerequisite): inst waits chain tail.
            if last_dma[cidx] is not None:
                tile.add_dep_helper(inst.ins, last_dma[cidx].ins, sync=True,
                                    reason="serialize DMA chain")
            last_dma[cidx] = inst
            return inst

        def x_chunk_dma(img, ch):
            base = ch * CR * W
            q = CR * W // 2
            for h in range(2):
                sl = slice(base + h * q, base + (h + 1) * q)
                chain(h, nc.sync.dma_start(x_sbs[img][:, :, sl],
                                           xr[img][:, :, sl]))

        def stage1_dma(img):
            x_alloc(img)
            for ch in range(N_CHUNK):
                x_chunk_dma(img, ch)

        def quant1_stage(src_ap, aq, blk, s_col, c_col, rows, y0):
            """Stage-1 quant, DVE only (bit-exact fp32 affine):
            a' = cast_fp8(min(max(src*s + coff, 12), 15)), coff = 3*bias+12;
            the fp8 cast in [12,15] rounds half-even == jnp.round."""
            t = t1pool.tile([128, rows * W], F32, tag="q1tmp")
            t3 = t[:].rearrange("p (r c) -> p r c", c=W)
            nc.vector.tensor_scalar(t3, src_ap, prm[:, s_col:s_col + 1],
                                    prm[:, c_col:c_col + 1], OP.mult, OP.add)
            nc.vector.tensor_scalar(aq_dst(aq, blk, rows, y0), t3,
                                    12.0, 15.0, OP.max, OP.min)

        def quant2_stage(src_ap, aq, blk, s_col, c_col, rows, y0):
            """Stage-2 quant: ACT affine+Relu (PSUM consumer) + DVE clamp."""
            t = t2pool.tile([128, rows * W], F32, tag="q2tmp")
            t3 = t[:].rearrange("p (r c) -> p r c", c=W)
            nc.scalar.activation(t3, src_ap, AF.Relu,
                                 scale=prm[:, s_col:s_col + 1],
                                 bias=prm[:, c_col:c_col + 1])
            nc.vector.tensor_scalar(aq_dst(aq, blk, rows, y0), t3,
                                    3.0, 12.0, OP.min, OP.add)

        def conv_tile(aq, w_sb, t, cb, row0=None, nrows=RT):
            ps = pspool.tile([128, nrows * W], F32, tag="ps")
            w5 = w_sb[:].rearrange("p (cb t j m) -> p cb t j m", cb=2, t=9,
                                   j=2)
            for tap in range(9):
                lhsT = w5[:, cb, tap]
                nc.tensor.matmul(ps[:], lhsT,
                                 aq_rhs(aq, t, tap, row0, nrows),
                                 perf_mode=mybir.MatmulPerfMode.DoubleRow,
                                 start=(tap == 0), stop=(tap == 8))
            return ps

        def stage1_units(img):
            """Quant thunks for image img, one per (chunk, blk)."""
            aq1 = aq1s[img % 2]
            x_sb = x_sbs[img]

            def make(ch, blk):
                def run():
                    sl = slice(ch * CR * W, (ch + 1) * CR * W)
                    src = x_sb[:, blk, sl].rearrange("p (r c) -> p r c", c=W)
                    quant1_stage(src, aq1, blk, 0 + blk, 2 + blk, CR, ch * CR)
                return run
            return [make(ch, blk) for ch in range(N_CHUNK) for blk in range(2)]

        def stage1_img0():
            """Image-0 stage-1 quant, latency-choreographed across ACT+DVE:
            ch0 h-split with the b1 affine on ACT (b0 on DVE), ch1/ch3
            h-split with both affines on ACT (a' = min(max(x*s+3b,0),3)+12,
            Relu on ACT then (min,add) on DVE -- the quant2 split); ch2 runs
            as full DVE units.  Both engines chase the DMA halves so the
            first tiles' aq rows are ready as the warmup matmuls end."""
            aq1, x_sb = aq1s[0], x_sbs[0]
            hh = CR // 2

            def src_of(ch, blk, r0, nr):
                sl = slice((ch * CR + r0) * W, (ch * CR + r0 + nr) * W)
                return x_sb[:, blk, sl].rearrange("p (r c) -> p r c", c=W)

            def act_aff(ch, blk, r0, nr):
                t = t2pool.tile([128, nr * W], F32, tag="q1a")
                t3 = t[:].rearrange("p (r c) -> p r c", c=W)
                nc.scalar.activation(t3, src_of(ch, blk, r0, nr), AF.Relu,
                                     scale=prm[:, 0 + blk:1 + blk],
                                     bias=prm[:, 10 + blk:11 + blk])
                return t3

            def dve_aff(ch, blk, r0, nr):
                t = t1pool.tile([128, nr * W], F32, tag="q1tmp")
                t3 = t[:].rearrange("p (r c) -> p r c", c=W)
                nc.vector.tensor_scalar(t3, src_of(ch, blk, r0, nr),
                                        prm[:, 0 + blk:1 + blk],
                                        prm[:, 2 + blk:3 + blk],
                                        OP.mult, OP.add)
                return t3

            def cl_f(t3, ch, blk, r0, nr):   # after DVE affine (+12 folded)
                nc.vector.tensor_scalar(aq_dst(aq1, blk, nr, ch * CR + r0),
                                        t3, 12.0, 15.0, OP.max, OP.min)

            def cl_r(t3, ch, blk, r0, nr):   # after ACT Relu affine
                nc.vector.tensor_scalar(aq_dst(aq1, blk, nr, ch * CR + r0),
                                        t3, 3.0, 12.0, OP.min, OP.add)

            # NOTE: the Tile scheduler reorders within queues, so DVE clamps
            # gated on ACT affines get hoisted unpredictably -- keep ch0/ch1
            # (first-tile critical) purely on DVE; only ch2 (which lands on
            # the scalar ring while DVE is busy) uses the ACT-affine split.
            for ch in (0, 1):
                for r0 in (0, hh):
                    for blk in range(2):
                        a0 = dve_aff(ch, blk, r0, hh)
                        cl_f(a0, ch, blk, r0, hh)
            for r0 in (0, hh):               # ch2: ACT affines, DVE clamps
                a0 = act_aff(2, 0, r0, hh)
                a1 = act_aff(2, 1, r0, hh)
                cl_r(a0, 2, 0, r0, hh)
                cl_r(a1, 2, 1, r0, hh)
            for r0 in (0, hh):               # ch3: full DVE half-units
                for blk in range(2):
                    a0 = dve_aff(3, blk, r0, hh)
                    cl_f(a0, 3, blk, r0, hh)

        def conv1_tile(img, t, cb):
            aq1, aq2 = aq1s[img % 2], aq2s[img % 2]
            ps = conv_tile(aq1, w1_sb, t, cb)
            psv = ps[:].rearrange("p (r c) -> p r c", c=W)
            quant2_stage(psv, aq2, cb, 4 + cb, 6 + cb, RT, t * RT)

        def conv2_tile(img, t, cb, split_epi=False):
            aq2, x_sb = aq2s[img % 2], x_sbs[img]
            ps = conv_tile(aq2, w2_sb, t, cb)
            psa = ps[:].rearrange("p (r c) -> p r c", c=W)
            # The very last tiles split their epilogue in half-row batches
            # (second half's out DMA on the scalar ring) to shorten the
            # post-final-matmul critical chain.
            nh = 2 if split_epi else 1
            rh = RT // nh
            for h in range(nh):
                psv = psa[:, h * rh:(h + 1) * rh]
                tt = t3pool.tile([128, rh * W], F32, tag="ot",
                                 name=f"ot_{img}_{t}_{cb}_{h}")
                tt3 = tt[:].rearrange("p (r c) -> p r c", c=W)
                nc.scalar.activation(tt3, psv, AF.Identity, scale=1.0 / 9.0,
                                     bias=prm[:, 8 + cb:9 + cb])
                lo = t * RT * W + h * rh * W
                res = x_sb[:, cb, lo: lo + rh * W]
                res3 = res.rearrange("p (r c) -> p r c", c=W)
                nc.vector.tensor_tensor(tt3, tt3, res3, OP.add)
                oq = nc.scalar if (split_epi and h % 2 == 1) else nc.sync
                oq.dma_start(outr[img][:, cb, lo: lo + rh * W], tt[:])

        def conv1_img(img, interleave=()):
            # Spread the next image's stage-1 units between conv1 tiles so
            # the DVE/ACT queues never head-of-line block the conv epilogues.
            inter = list(interleave)
            for t in range(NT):
                for cb in range(2):
                    conv1_tile(img, t, cb)
                for _ in range(2):
                    if inter:
                        inter.pop(0)()
            for f in inter:
                f()

        def conv2_rows(img, row0, nrows, cb, oq):
            """Row-range conv2 tile + one-shot epilogue, out DMA on oq."""
            aq2, x_sb = aq2s[img % 2], x_sbs[img]
            ps = conv_tile(aq2, w2_sb, None, cb, row0=row0, nrows=nrows)
            psv = ps[:].rearrange("p (r c) -> p r c", c=W)
            tt = t3pool.tile([128, nrows * W], F32, tag="ot",
                             name=f"otr_{img}_{row0}_{cb}")
            tt3 = tt[:].rearrange("p (r c) -> p r c", c=W)
            nc.scalar.activation(tt3, psv, AF.Identity, scale=1.0 / 9.0,
                                 bias=prm[:, 8 + cb:9 + cb])
            lo = row0 * W
            res3 = x_sb[:, cb, lo: lo + nrows * W].rearrange(
                "p (r c) -> p r c", c=W)
            nc.vector.tensor_tensor(tt3, tt3, res3, OP.add)
            oq.dma_start(outr[img][:, cb, lo: lo + nrows * W], tt[:])

        def conv2_img(img, last=False):
            for t in range(NT):
                for cb in range(2):
                    if last and t == NT - 1 and cb == 1:
                        # final cout block ends on a 2-row tile so the
                        # exposed post-stream epilogue chain is minimal
                        conv2_rows(img, (NT - 1) * RT, RT - 2, 1, nc.sync)
                        conv2_rows(img, H - 2, 2, 1, nc.scalar)
                    else:
                        conv2_tile(img, t, cb,
                                   split_epi=(last and t == NT - 1))

        # --- startup: concurrent rings split the same engine bandwidth, so
        # everything latency-critical rides the sync ring in one FIFO with
        # prm + the w1 halves at its head (the v0 all-on-sync feed held a
        # steady ~2.9us/chunk x cadence; split-ring variants degraded it).
        # Only w2 uses the scalar ring, dep-held until chunk 2 has landed.
        nc.sync.dma_start(prm[:], prm_d.ap())
        nc.sync.dma_start(w1_sb[:, 0:2304], w1_d.ap()[:, 0:2304])
        x_alloc(0)
        x_chunk_dma(0, 0)
        # w1's cb1 half slots in after chunk 0 (needed ~1.7us after the
        # first tile; chunk 0's h1 half gates the stream start instead)
        nc.sync.dma_start(w1_sb[:, 2304:4608], w1_d.ap()[:, 2304:4608])
        for ch in range(1, N_CHUNK):
            x_chunk_dma(0, ch)
        w2i = nc.scalar.dma_start(w2_sb[:], w2_d.ap())
        tile.add_dep_helper(w2i.ins, last_dma[1].ins, sync=True,
                            reason="hold w2 until the startup x feed clears")

        wu_ps = pspool.tile([128, TQ], F32, tag="ps")
        for i in range(N_WARMUP):
            nc.tensor.matmul(wu_ps[:], wu_src[:, :, 0:128],
                             wu_src[:, :, 0:TQ],
                             perf_mode=mybir.MatmulPerfMode.DoubleRow,
                             start=(i == 0), stop=(i == N_WARMUP - 1))

        stage1_img0()
        if N_IMG > 1:
            stage1_dma(1)
            conv1_img(0, interleave=stage1_units(1))
        else:
            conv1_img(0)
        for img in range(1, N_IMG):
            nxt = ()
            if img + 1 < N_IMG:
                stage1_dma(img + 1)
                nxt = stage1_units(img + 1)
            conv1_img(img, interleave=nxt)
            conv2_img(img - 1)
        conv2_img(N_IMG - 1, last=True)

    nc.compile()
    return nc


def _host_prep(w1, w2, g1, b1, m1, v1, g2, b2, m2, v2):
    """BN folds + DoReFa weight quantization, replicating the reference's
    fp32 op sequence exactly (jax CPU), then weight layout transforms."""
    import jax
    import jax.numpy as jnp
    import ml_dtypes

    cpu = jax.local_devices(backend="cpu")[0]
    with jax.default_device(cpu):
        eps = jnp.float32(1e-5)
        inv1 = g1 / jnp.sqrt(v1 + eps)
        bias1 = b1 - m1 * inv1
        inv2 = g2 / jnp.sqrt(v2 + eps)
        bias2 = b2 - m2 * inv2

        def wq3(w):
            wt = jnp.tanh(w)
            wn = wt / (2.0 * jnp.max(jnp.abs(wt))) + 0.5
            return 2.0 * jnp.round(wn * 3.0) - 3.0   # exact ints {-3,-1,1,3}

        wq1 = np.asarray(wq3(jnp.asarray(w1)), dtype=np.float32)
        wq2 = np.asarray(wq3(jnp.asarray(w2)), dtype=np.float32)
        inv1, bias1, inv2, bias2 = (np.asarray(a, dtype=np.float32)
                                    for a in (inv1, bias1, inv2, bias2))

    S1 = wq1.reshape(256, -1).sum(axis=1).astype(np.float32)
    S2 = wq2.reshape(256, -1).sum(axis=1).astype(np.float32)

    s1 = 3.0 * inv1
    c1 = 3.0 * bias1 + 12.0      # stage-1 offset folded into the affine
    s2 = inv2 / np.float32(3.0)
    c2 = 3.0 * bias2 - 4.0 * S1 * inv2
    corr2 = -(np.float32(4.0) / np.float32(3.0)) * S2

    def wlayout(wq):
        # [cout, cin, ky, kx] -> [k(128), cb(2), tap(9), blk(2), m(128)]
        a = wq.reshape(2, 128, 2, 128, 9)             # cb, m, blk, k, tap
        return np.ascontiguousarray(np.transpose(a, (3, 0, 4, 2, 1))
                                    .reshape(128, 4608)
                                    ).astype(ml_dtypes.float8_e4m3)

    c1m12 = 3.0 * bias1                 # c1 - 12, for the ACT-Relu variant

    prm = np.zeros((128, 12), np.float32)
    for col, v in enumerate((s1, c1, s2, c2, corr2, c1m12)):
        prm[:, 2 * col] = v[0:128]
        prm[:, 2 * col + 1] = v[128:256]

    return {"w1t": wlayout(wq1), "w2t": wlayout(wq2), "prm": prm}


def kernel(x, w1, w2, g1, b1, m1, v1, g2, b2, m2, v2):
    global LAST_EXEC_NS
    x = np.asarray(x, dtype=np.float32)

    if "nc" not in _CACHED:
        _CACHED["nc"] = _build()
    nc = _CACHED["nc"]

    shared = _host_prep(w1, w2, g1, b1, m1, v1, g2, b2, m2, v2)
    in_maps = []
    for c in range(N_CORES):
        m = dict(shared)
        m["x"] = x[N_IMG * c:N_IMG * (c + 1)]
        in_maps.append(m)

    trace = bool(int(os.environ.get("BASS_TRACE", "0")))
    res = run_bass_kernel_spmd(nc, in_maps, core_ids=list(range(N_CORES)),
                               trace=trace)
    LAST_EXEC_NS = res.exec_time_ns
    return np.concatenate([res.results[c]["out"] for c in range(N_CORES)],
                          axis=0)

